# revision 19
# baseline (speedup 1.0000x reference)
"""Epipolar attention kernel for Trainium2 (8 NeuronCores, batch-parallel).

Host does the O(B) 3x3 geometry (SVD etc.) in float32 numpy, mirroring the
reference op-for-op; the device does all O(N^2) / O(N^2*C) work:
  d5[i,j]  = A_j*x_i + B_j*y_i + C_j          (PE, triple-bf16 split)
  dabs     = |d5| (fp16), m_i = rowmax        (DVE)
  e        = exp(dabs - m_i), r_i = rowsum    (ACT, fused accum)
  E2[i,j]  = exp(-e/r_i)                      (ACT, fp16)
  S_j      = colsum_i E2                      (PE ones-matmul)
  at8[j,i] = fp8(sigma*(E2^T/S_j - 1/N))      (PE transpose + ACT/DVE quant)
  out[i,c] = mean_j(V)[c] + (1/sigma)*sum_j at8[j,i]*V8[j,c]
                                              (PE fp8 DoubleRow + DVE evac)
The rank-1 split works because colmean_i(attn) == 1/N exactly (softmax over
i sums to 1), leaving a tiny delta that fp8 carries with ~1e-3 total error.
"""

import numpy as np
import ml_dtypes

import concourse.bass as bass
import concourse.bacc as bacc
import concourse.tile as tile
from concourse import mybir
from concourse.bass_utils import run_bass_kernel_spmd

B, C, H, W = 8, 1152, 32, 32
N = H * W           # 1024
P = 128
NT = N // P         # 8
NQ = 4              # fp8 DoubleRow k-chunks of 256
F32 = mybir.dt.float32
F16 = mybir.dt.float16
BF16 = mybir.dt.bfloat16
F8 = mybir.dt.float8e4
BFNP = ml_dtypes.bfloat16
F8NP = ml_dtypes.float8_e4m3
SIGMA = float(2.0 ** 17)

TRACE = False
LAST_RESULTS = None


# ----------------------------------------------------------------- device ---

def _build_nc():
    nc = bacc.Bacc()
    xyabc = nc.dram_tensor("xyabc", (9, 2 * N), BF16, kind="ExternalInput")
    identD = nc.dram_tensor("ident", (P, P), F16, kind="ExternalInput")
    v8d = nc.dram_tensor("v8d", (P, NQ * 2 * C), F8, kind="ExternalInput")
    basebD = nc.dram_tensor("baseb", (P, C), F16, kind="ExternalInput")
    out = nc.dram_tensor("out", (N, C), F16, kind="ExternalOutput")
    I32 = mybir.dt.int32

    AF = mybir.ActivationFunctionType
    AO = mybir.AluOpType
    PM = mybir.MatmulPerfMode

    with tile.TileContext(nc) as tc:
        with (
            tc.tile_pool(name="consts", bufs=1) as consts,
            tc.tile_pool(name="persist", bufs=1) as persist,
            tc.tile_pool(name="work", bufs=3) as work,
            tc.tile_pool(name="stats", bufs=4) as stats,
        ):
            xyabc_sb = consts.tile([9, 2 * N], BF16, tag="xyabc")
            nc.sync.dma_start(out=xyabc_sb, in_=xyabc[:, :])
            ident = consts.tile([P, P], F16, tag="ident")
            nc.sync.dma_start(out=ident, in_=identD[:, :])
            v8_sb = consts.tile([P, NQ, 2, C], F8, tag="v8")
            nc.sync.dma_start(out=v8_sb, in_=v8d[:, :])
            ones8 = consts.tile([P, 8], F16, tag="ones8")
            nc.vector.memset(ones8, 1.0)
            baseb_sb = consts.tile([P, C], F16, tag="baseb")
            nc.sync.dma_start(out=baseb_sb, in_=basebD[:, :])

            e2_sb = persist.tile([P, NT, N], F16, tag="e2")
            at8_sb = persist.tile([P, NQ, 2, N], F8, tag="at8")

            with (
                tc.tile_pool(name="psS", bufs=1, space="PSUM") as psS,
                tc.tile_pool(name="psAB", bufs=1, space="PSUM") as psAB,
            ):
                srow_ps = psS.tile([8, N], F32, tag="srow")

                # Phase A: rows i on partitions, j on free dim.
                # exp2(it) is emitted after exp1(it+1): the Scalar engine
                # has no exec queue, so exp2's wait on the reciprocal
                # round-trip would stall the next tile's exp1 otherwise.
                def emit_exp2(it, e_t, ninvr):
                    nc.scalar.activation(
                        out=e2_sb[:, it, :], in_=e_t, func=AF.Exp, bias=0.0,
                        scale=ninvr,
                    )
                    # column sums: srow[0:8,j] = sum_i E2[i,j] (8 dup rows)
                    for h in range(2):
                        nc.tensor.matmul(
                            srow_ps[:, h * 512:(h + 1) * 512],
                            lhsT=ones8,
                            rhs=e2_sb[:, it, h * 512:(h + 1) * 512],
                            start=(it == 0), stop=(it == NT - 1),
                        )

                pend = None
                for it in range(NT):
                    d_ps = psAB.tile([P, N], F32, tag="d", bufs=2)
                    for h in range(2):
                        nc.tensor.matmul(
                            d_ps[:, h * 512:(h + 1) * 512],
                            lhsT=xyabc_sb[:, it * P:(it + 1) * P],
                            rhs=xyabc_sb[:, N + h * 512:N + (h + 1) * 512],
                            start=True, stop=True,
                        )
                    nmx = stats.tile([P, 1], F32, tag="nmx")
                    nc.vector.tensor_reduce(
                        out=nmx, in_=d_ps, axis=mybir.AxisListType.X,
                        op=AO.max, apply_absolute_value=True, negate=True,
                    )
                    dabs = work.tile([P, N], F32, tag="dabs")
                    nc.vector.tensor_scalar(
                        out=dabs.bitcast(I32), in0=d_ps.bitcast(I32),
                        scalar1=0x7FFFFFFF, scalar2=None, op0=AO.bitwise_and,
                    )
                    e_t = work.tile([P, N], F16, tag="et")
                    r = stats.tile([P, 1], F32, tag="r")
                    nc.scalar.activation(
                        out=e_t, in_=dabs, func=AF.Exp, bias=nmx, scale=1.0,
                        accum_out=r,
                    )
                    negr = stats.tile([P, 1], F32, tag="negr")
                    nc.gpsimd.tensor_scalar_mul(negr, r, -1.0)
                    ninvr = stats.tile([P, 1], F32, tag="ninvr")
                    nc.vector.reciprocal(ninvr, negr)     # -1/r
                    if pend is not None:
                        emit_exp2(*pend)
                    pend = (it, e_t, ninvr)
                emit_exp2(*pend)

                # Phase B: S stats, transposes, fp8 delta quantization
                srow_sb = stats.tile([8, N], F16, tag="srow_sb")
                nc.scalar.copy(srow_sb, srow_ps[0:8, :])
                scolT_ps = psAB.tile([P, 8, 8], F16, tag="d", bufs=2)
                for u in range(NT):
                    nc.tensor.transpose(
                        scolT_ps[:, u, :], srow_sb[0:8, u * P:(u + 1) * P],
                        ident[0:8, 0:8],
                    )
                scol = stats.tile([P, 8], F32, tag="scol")
                nc.vector.tensor_scalar(
                    out=scol, in0=scolT_ps[:, :, 0], scalar1=0.0, scalar2=None,
                    op0=AO.add,
                )
                sginv = stats.tile([P, 8], F32, tag="sginv")
                nc.vector.reciprocal(sginv, scol)         # 1/S
                nc.vector.tensor_scalar_mul(sginv, sginv, SIGMA)  # sigma/S

                for u in range(NT):
                    tp = psAB.tile([P, N], F16, tag="tp", bufs=2)
                    q, hh = divmod(u, 2)
                    for half in range(2):
                        for it in range(half * 4, half * 4 + 4):
                            nc.tensor.transpose(
                                tp[:, it * P:(it + 1) * P],
                                e2_sb[:, it, u * P:(u + 1) * P],
                                ident,
                            )
                        sl = slice(half * 512, (half + 1) * 512)
                        if (u + half) % 2 == 0:
                            nc.scalar.activation(
                                out=at8_sb[:, q, hh, sl], in_=tp[:, sl],
                                func=AF.Copy, bias=-SIGMA / N,
                                scale=sginv[:, u:u + 1],
                            )
                        else:
                            nc.vector.tensor_scalar(
                                out=at8_sb[:, q, hh, sl], in0=tp[:, sl],
                                scalar1=sginv[:, u:u + 1], scalar2=-SIGMA / N,
                                op0=AO.mult, op1=AO.add,
                            )

            # Phase C: out[i,c] = base[c] + (1/sigma) sum_j at8[j,i] v8[j,c]
            CCH = ((0, 512), (512, 512), (1024, 128))
            with tc.tile_pool(name="psC", bufs=2, space="PSUM") as psC:
                for it in range(NT):
                    ocs = [psC.tile([P, cw], F32, tag=f"oc{ci}",
                                    name=f"oc_{it}_{ci}",
                                    bufs=(2 if ci == 2 else 3))
                           for ci, (c0, cw) in enumerate(CCH)]
                    for q in range(NQ):
                        for ck, (c0, cw) in enumerate(CCH):
                            nc.tensor.matmul(
                                ocs[ck][:, :],
                                lhsT=at8_sb[:, q, :, it * P:(it + 1) * P],
                                rhs=v8_sb[:, q, :, c0:c0 + cw],
                                start=(q == 0), stop=(q == NQ - 1),
                                perf_mode=PM.DoubleRow,
                            )
                    osb = work.tile([P, C], F16, tag="osb")
                    for ck, (c0, cw) in enumerate(CCH):
                        nc.vector.scalar_tensor_tensor(
                            out=osb[:, c0:c0 + cw], in0=ocs[ck],
                            scalar=1.0 / SIGMA, in1=baseb_sb[:, c0:c0 + cw],
                            op0=AO.mult, op1=AO.add,
                        )
                    nc.gpsimd.dma_start(
                        out=out[it * P:(it + 1) * P, :], in_=osb,
                    )
    nc.compile()
    return nc


_NC = None


def _get_nc():
    global _NC
    if _NC is None:
        _NC = _build_nc()
    return _NC


# ------------------------------------------------------------------- host ---

def _skew(t):
    z = np.zeros_like(t[:, 0])
    return np.stack([
        np.stack([z, -t[:, 2], t[:, 1]], -1),
        np.stack([t[:, 2], z, -t[:, 0]], -1),
        np.stack([-t[:, 1], t[:, 0], z], -1),
    ], 1)


def _fundamental(K1, K2, R, t):
    E = _skew(t) @ R
    U, S, Vt = np.linalg.svd(E)
    S = S.copy()
    S[:, 2] = 0.0
    E = U @ (S[:, :, None] * Vt)
    return np.linalg.inv(np.swapaxes(K2, 1, 2)) @ E @ np.linalg.inv(K1)


def _split3(v):
    """Exact-ish triple bf16 split: v ~= hi + mid + lo (24 mantissa bits)."""
    v = v.astype(np.float32)
    hi = v.astype(BFNP)
    r1 = v - hi.astype(np.float32)
    mid = r1.astype(BFNP)
    r2 = r1 - mid.astype(np.float32)
    lo = r2.astype(BFNP)
    return hi, mid, lo


def _host_prep(f_src, K1, K2, R, t):
    ix, iy = np.meshgrid(np.arange(H, dtype=np.float32),
                         np.arange(W, dtype=np.float32), indexing="ij")
    comb = np.stack([ix.ravel(), iy.ravel(), np.ones(N, np.float32)], 0)  # (3,N)

    F = _fundamental(K1, K2, R, t)                    # (B,3,3)
    lines = (F @ comb).astype(np.float32)             # (B,3,N)
    lines = lines / lines[:, 2:3, :]
    y0 = -lines[:, 2, :] / lines[:, 1, :]
    y1 = -(lines[:, 2, :] + lines[:, 0, :] * np.float32(W)) / lines[:, 1, :]
    dy = y0 - y1
    L = np.sqrt(np.float32(W * W) + dy * dy)
    A5 = np.float32(5.0) * (dy / L)
    B5 = np.float32(5.0) * (np.float32(W) / L)
    C5 = np.float32(-5.0) * (np.float32(W) * y0 / L)

    Ah, Am, Al = _split3(A5)
    Bh, Bm, Bl = _split3(B5)
    Ch, Cm, Cl = _split3(C5)
    abc9 = np.stack([Ah, Bh, Ch, Am, Bm, Cm, Al, Bl, Cl], axis=1)  # (B,9,N)
    xy9 = np.tile(comb, (3, 1)).astype(BFNP)                        # (9,N)
    xyabc = np.concatenate(
        [np.broadcast_to(xy9, (B, 9, N)), abc9], axis=2)            # (B,9,2N)

    V = f_src.reshape(B, C, N).transpose(0, 2, 1)     # (B,N,C) fp32
    # v8[b, p, q, h, c] = fp8(V[b, 256q+128h+p, c])
    v8 = np.ascontiguousarray(
        V.reshape(B, NQ, 2, P, C).transpose(0, 3, 1, 2, 4)).astype(F8NP)
    base = V.mean(axis=1, dtype=np.float64).astype(np.float32)      # (B,C)
    baseb = np.broadcast_to(
        base[:, None, :].astype(np.float16), (B, P, C))             # (B,P,C)
    return xyabc, v8, baseb


def kernel(f_tar=None, f_src=None, K1=None, K2=None, R=None, t=None):
    global LAST_RESULTS
    f_src = np.asarray(f_src, np.float32)
    K1 = np.asarray(K1, np.float32)
    K2 = np.asarray(K2, np.float32)
    R = np.asarray(R, np.float32)
    t = np.asarray(t, np.float32)

    xyabc, v8, baseb = _host_prep(f_src, K1, K2, R, t)
    ident = np.eye(P, dtype=np.float16)
    in_maps = [
        {"xyabc": np.ascontiguousarray(xyabc[b]),
         "ident": ident,
         "v8d": v8[b].reshape(P, NQ * 2 * C),
         "baseb": np.ascontiguousarray(baseb[b])}
        for b in range(B)
    ]
    res = run_bass_kernel_spmd(_get_nc(), in_maps, list(range(B)), trace=TRACE)
    LAST_RESULTS = res
    outs = np.stack([res.results[b]["out"].astype(np.float32)
                     for b in range(B)], 0)           # (B,N,C)
    return outs.reshape(B, C, H, W)
